# revision 1
# baseline (speedup 1.0000x reference)
"""Weighted-KNN (retrieval_knn) Trainium2 kernel.

Math (per query c, over N anchors):
    sq[n]   = ||c - p_n||^2 / (w_n^2 + eps)
    top-8 smallest sq -> softmax(-sq_k / TEMP) -> weighted sum of features.

Device strategy (per core, data-parallel over B across 8 cores):
  * y[q, n] = -sq[q,n]/TEMP computed on TensorE as a rank-5 inner product
    over centered coordinates (c' = c - 0.5, p' = p - 0.5):
        h_q = [||c'_q||^2, c'_q0, c'_q1, c'_q2, 1]
        g_n = (-1/TEMP) * inv_n * [1, -2p'_n0, -2p'_n1, -2p'_n2, ||p'_n||^2]
    The anchor axis is split into 4 groups handled by 4 concurrent
    row-tiled matmuls (tile_position=(32m, 0)); each group's 5 G rows and
    5 H rows live on disjoint 32-aligned partition lanes, so the four
    fp32 streams overlap on the PE array.
  * Packed top-8 trick: the PSUM->SBUF copy writes y as bf16 into the HIGH
    halves of persistent fp32 words whose LOW halves hold the (half-
    relative) anchor index, initialized once.  fp32 `max` (DVE top-8) on
    the packed words yields value ordering AND the index in one pass --
    no max_index scan.
  * Top-8 of each N/2 half (the union provably contains the true top-8:
    any true top-8 member has at most 7 better anchors anywhere).
  * The 16 candidates are re-scored EXACTLY from gathered [p', -inv/TEMP]
    rows (direct differences, no cancellation); top-8 selection + softmax
    run on the exact scores, so bf16/matmul rounding only perturbs
    candidates around global rank ~16 whose weights are negligible.
  * Feature rows fetched with gpsimd dma_gather; masked softmax-weighted
    sum on VectorE.
"""

import sys

if "/opt/trn_rl_repo" not in sys.path:
    sys.path.insert(0, "/opt/trn_rl_repo")

import numpy as np

import concourse.bacc as bacc
import concourse.bass as bass
import concourse.mybir as mybir
from concourse.bass import ts
from concourse.bass_utils import run_bass_kernel_spmd
from concourse.tile import TileContext

B, N, D, F = 65536, 16384, 3, 64
K = 8
BANDWIDTH = 0.05
TEMP = 2.0 * BANDWIDTH * BANDWIDTH  # 0.005
INV_TEMP = 1.0 / TEMP  # 200.0
EPS = 1e-8
NCORES = 8
Q = B // NCORES  # 8192 queries per core
P = 128
NT = Q // P  # 64 query tiles per core
CH = 512  # matmul free-dim chunk
NG = 4  # row-tiled matmul groups
NGN = N // NG  # 4096 anchors per group
NCHG = NGN // CH  # 8 chunk-steps
NH = N // NG  # 4096 anchors per quarter(=group)
NC = NG * K  # 32 candidates per query
NPK = 6  # packed-y buffer rotation depth
LOOP = 1  # in-NEFF repetitions of the whole tile loop (benchmarking)
STAGE = 99  # bench bisect: 1=mm+max8, 2=+perm/idxw, 3=+gathers, 99=full

FP = mybir.dt.float32
BF = mybir.dt.bfloat16
I32 = mybir.dt.int32


def _build_nc():
    nc = bacc.Bacc("TRN2", num_swdge_queues=2)
    coords = nc.declare_dram_parameter("coords", [Q, D], FP, isOutput=False)
    positions = nc.declare_dram_parameter("positions", [N, D], FP, isOutput=False)
    weights = nc.declare_dram_parameter("weights", [N], FP, isOutput=False)
    features = nc.declare_dram_parameter("features", [N, F], FP, isOutput=False)
    ident_in = nc.declare_dram_parameter("ident", [P, P], FP, isOutput=False)
    perm_in = nc.declare_dram_parameter("perm", [P, 8, P], FP, isOutput=False)
    pkinit_in = nc.declare_dram_parameter("pkinit", [P, NH], I32, isOutput=False)
    out = nc.declare_dram_parameter("out", [Q, F], FP, isOutput=True)

    # combined gather table: row n = [features(64) | p'_0 p'_1 p'_2 -inv/TEMP | pad]
    comb_hbm = nc.dram_tensor("comb_stage", [N, 2 * F], FP)

    with TileContext(nc) as tc:
        with (
            tc.tile_pool(name="const", bufs=1) as cpool,
            nc.gpsimd.register("nidx") as nidx_reg,
        ):
            nc.gpsimd.reg_mov(nidx_reg, P * K)

            ident = cpool.tile([P, P], FP)
            nc.sync.dma_start(ident[:], ident_in[:])
            pconst = cpool.tile([P, 8, P], FP)
            nc.sync.dma_start(pconst[:], perm_in[:])

            # G4[32m + r, j] = g_r[m*4096 + j]  (4 groups on partition lanes)
            G4 = cpool.tile([P, NGN], FP)

            # persistent packed-y buffers; low halves = half-relative idx
            pk = [
                cpool.tile([P, NH], FP, name=f"pk{i}", tag=f"pk{i}")
                for i in range(NPK)
            ]
            for i in range(NPK):
                nc.sync.dma_start(pk[i][:].bitcast(I32), pkinit_in[:])

            # ---------------- prep: build G and the rescore table ----------------
            with tc.tile_pool(name="prep", bufs=2) as pp:
                # anchors laid out n = 128*p + j
                pos_sb = pp.tile([P, P, D], FP)
                nc.sync.dma_start(
                    pos_sb[:], positions[:].rearrange("(p j) d -> p j d", p=P)
                )
                # center: p' = p - 0.5
                nc.vector.tensor_scalar_add(pos_sb[:], pos_sb[:], -0.5)
                w_sb = pp.tile([P, P], FP)
                nc.sync.dma_start(w_sb[:], weights[:].rearrange("(p j) -> p j", p=P))

                inv = pp.tile([P, P], FP)
                nc.vector.tensor_mul(inv[:], w_sb[:], w_sb[:])
                nc.vector.tensor_scalar_add(inv[:], inv[:], EPS)
                nc.vector.reciprocal(inv[:], inv[:])

                g0 = pp.tile([P, P], FP)
                nc.vector.tensor_scalar_mul(g0[:], inv[:], -INV_TEMP)

                gd = [
                    pp.tile([P, P], FP, tag=f"g{d + 1}", name=f"g{d + 1}")
                    for d in range(D)
                ]
                for d in range(D):
                    nc.vector.tensor_mul(gd[d][:], inv[:], pos_sb[:, :, d])
                    nc.vector.tensor_scalar_mul(gd[d][:], gd[d][:], 2.0 * INV_TEMP)

                pp2 = pp.tile([P, P], FP)
                tmp = pp.tile([P, P], FP)
                nc.vector.tensor_mul(pp2[:], pos_sb[:, :, 0], pos_sb[:, :, 0])
                nc.vector.tensor_mul(tmp[:], pos_sb[:, :, 1], pos_sb[:, :, 1])
                nc.vector.tensor_add(pp2[:], pp2[:], tmp[:])
                nc.vector.tensor_mul(tmp[:], pos_sb[:, :, 2], pos_sb[:, :, 2])
                nc.vector.tensor_add(pp2[:], pp2[:], tmp[:])
                g4c = pp.tile([P, P], FP)
                nc.vector.tensor_mul(g4c[:], g0[:], pp2[:])

                # scatter [128, 128] component tiles into G4 group lanes:
                # comp_r partitions [32m, 32m+32) hold n in [4096m, 4096m+4096)
                for r, comp in enumerate([g0, gd[0], gd[1], gd[2], g4c]):
                    for m in range(NG):
                        src = comp[32 * m : 32 * (m + 1), :]
                        dst = bass.AP(
                            G4[:].tensor,
                            (32 * m + r) * NGN,
                            [[NGN, 1], [P, 32], [1, P]],
                        )
                        nc.sync.dma_start(dst, src)

                # rescore table: interleave [p'0, p'1, p'2, g0] per anchor,
                # DMA'd (chunked) into the first 16 bytes of each 256B row.
                pwt = pp.tile([P, P, 4], FP)
                for f, comp in enumerate(
                    [pos_sb[:, :, 0], pos_sb[:, :, 1], pos_sb[:, :, 2], g0[:]]
                ):
                    nc.vector.tensor_copy(pwt[:, :, f], comp)
                pw_rows = comb_hbm[:, F : F + 4].rearrange("(p j) f -> p j f", p=P)
                for ck in range(8):
                    nc.sync.dma_start(
                        pw_rows[ts(ck, 16), :, :], pwt[ts(ck, 16), :, :]
                    )
                # features -> comb rows (HBM->HBM, chunked)
                feat_rows = comb_hbm[:, 0:F].rearrange("(a n) f -> a n f", a=8)
                src_rows = features[:].rearrange("(a n) f -> a n f", a=8)
                for ck in range(8):
                    nc.sync.dma_start(feat_rows[ck, :, :], src_rows[ck, :, :])

            # ---------------- main loop over query tiles ----------------
            with (
                tc.tile_pool(name="mm_ps", bufs=6, space="PSUM") as pspool,
                tc.tile_pool(name="ht_ps", bufs=2, space="PSUM") as htpool,
                tc.tile_pool(name="hs", bufs=4) as hpool,
                tc.tile_pool(name="sm", bufs=6) as sm,
                tc.tile_pool(name="g8", bufs=3) as gpool,
            ):
                for tl in range(NT * LOOP):
                    t = tl % NT
                    # --- per-tile H build, replicated to 4 group lanes ---
                    ct = hpool.tile([P, D], FP, tag="ct")
                    nc.sync.dma_start(ct[:], coords[ts(t, P), :])
                    nc.vector.tensor_scalar_add(ct[:], ct[:], -0.5)
                    nct = hpool.tile([P, D], FP, tag="nct")
                    nc.vector.tensor_scalar_mul(nct[:], ct[:], -1.0)
                    cc = hpool.tile([P, D], FP, tag="cc")
                    nc.vector.tensor_mul(cc[:], ct[:], ct[:])
                    hsrc = hpool.tile([P, 5], FP, tag="hsrc")
                    nc.vector.reduce_sum(
                        out=hsrc[:, 0:1], in_=cc[:], axis=mybir.AxisListType.X
                    )
                    nc.vector.tensor_copy(hsrc[:, 1:4], ct[:])
                    nc.vector.memset(hsrc[:, 4:5], 1.0)
                    psT = htpool.tile([P, P], FP, tag="htmp")
                    nc.tensor.transpose(psT[:5, :], hsrc[:], ident[:])
                    hT4 = hpool.tile([P, P], FP, tag="hT4")
                    for m in range(NG):
                        nc.scalar.copy(hT4[32 * m : 32 * m + 5, :], psT[:5, :])

                    # --- distances: 4 concurrent row-tiled matmuls/step ---
                    pkb = [pk[(NG * tl + m) % NPK] for m in range(NG)]
                    for c in range(NCHG):
                        for m in range(NG):
                            ps = pspool.tile(
                                [P, CH], FP, tag="ps", name=f"ps{tl}_{c}_{m}"
                            )
                            nc.tensor.matmul(
                                ps[:],
                                hT4[32 * m : 32 * m + 5, :],
                                G4[32 * m : 32 * m + 5, ts(c, CH)],
                                start=True,
                                stop=True,
                                tile_position=(32 * m, 0),
                            )
                            # bf16 y into high halves of group m's words
                            hi = bass.AP(
                                pkb[m][:].bitcast(BF).tensor,
                                2 * CH * c + 1,
                                [[2 * NH, P], [2, CH]],
                            )
                            nc.scalar.copy(hi, ps[:])

                    # --- packed top-8 per quarter; extract indices ---
                    idx32 = sm.tile([P, NC], I32, tag="idx32")
                    for m in range(NG):
                        v8p = sm.tile([P, K], FP, tag="v8p", name=f"v8p_{tl}_{m}")
                        nc.vector.max(v8p[:], pkb[m][:])
                        nc.vector.tensor_scalar(
                            idx32[:, ts(m, K)],
                            v8p[:].bitcast(I32),
                            65535,
                            None,
                            op0=mybir.AluOpType.bitwise_and,
                        )
                        if m:
                            nc.vector.tensor_scalar_add(
                                idx32[:, ts(m, K)], idx32[:, ts(m, K)], float(m * NH)
                            )

                    if STAGE == 1:
                        dump = sm.tile([P, F], FP, tag="dump", name=f"dump{tl}")
                        nc.vector.tensor_copy(dump[:, 0:NC], idx32[:])
                        nc.sync.dma_start(out[ts(t, P), :], dump[:])
                        continue

                    # --- wrapped int16 idx layout for dma_gather:
                    # idxw[16g+p, 8k+u] = idx[16u+p, k] via 8 permutation
                    # matmuls (perm_u[q, p'] = 1 iff q == 16u + p'%16).
                    idxf = sm.tile([P, NC], FP, tag="idxf")
                    nc.vector.tensor_copy(idxf[:], idx32[:])
                    psI = htpool.tile([P, 8, NC], FP, tag="htmp", name=f"psI_{tl}")
                    for u in range(8):
                        nc.tensor.matmul(
                            psI[:, u, :],
                            pconst[:, u, :],
                            idxf[:],
                            start=True,
                            stop=True,
                        )
                    idxw = sm.tile([P, NC * 8], mybir.dt.int16, tag="idxw")
                    idxw_uk = bass.AP(
                        idxw[:].tensor, 0, [[NC * 8, P], [1, 8], [8, NC]]
                    )
                    nc.vector.tensor_copy(idxw_uk, psI[:])

                    if STAGE == 2:
                        dump = sm.tile([P, F], FP, tag="dump", name=f"dump{tl}")
                        nc.vector.tensor_copy(dump[:], idxw[:, 0:128].bitcast(FP))
                        nc.sync.dma_start(out[ts(t, P), :], dump[:])
                        continue

                    # --- gather candidate rescore rows + feature rows ---
                    cg = gpool.tile([P, NC, 2 * F], FP, tag="cg")
                    for m in range(NG):
                        isl = idxw[:, m * K * 8 : (m + 1) * K * 8]
                        nc.gpsimd.dma_gather(
                            cg[:, m * K : (m + 1) * K, :],
                            comb_hbm[:],
                            isl,
                            P * K,
                            nidx_reg,
                            2 * F,
                            queue_num=m % 2,
                        )
                    g32 = cg[:, :, 0:F]
                    pwg = cg[:, :, F : 2 * F]

                    if STAGE == 3:
                        dump = sm.tile([P, F], FP, tag="dump", name=f"dump{tl}")
                        nc.vector.tensor_add(dump[:], cg[:, 0, 0:F], cg[:, 0, F:2*F])
                        nc.sync.dma_start(out[ts(t, P), :], dump[:])
                        continue

                    # --- exact rescore: y32 = sum_d (p'_d - c'_d)^2 * g0 ---
                    sqd = [
                        sm.tile([P, NC], FP, tag=f"sqd{d}", name=f"sqd{d}")
                        for d in range(D)
                    ]
                    for d in range(D):
                        nc.scalar.activation(
                            sqd[d][:],
                            pwg[:, :, d],
                            mybir.ActivationFunctionType.Square,
                            bias=nct[:, d : d + 1],
                            scale=1.0,
                        )
                    nc.vector.tensor_add(sqd[0][:], sqd[0][:], sqd[1][:])
                    nc.vector.tensor_add(sqd[0][:], sqd[0][:], sqd[2][:])
                    y16 = sm.tile([P, NC], FP, tag="y16")
                    nc.vector.tensor_mul(y16[:], sqd[0][:], pwg[:, :, 3])

                    # --- exact top-8 + masked softmax over 16 candidates ---
                    v8x = sm.tile([P, K], FP, tag="v8x")
                    nc.vector.max(v8x[:], y16[:])
                    nv1 = sm.tile([P, 1], FP, tag="nv1")
                    nc.vector.tensor_scalar_mul(nv1[:], v8x[:, 0:1], -1.0)
                    e16 = sm.tile([P, NC], FP, tag="e16")
                    nc.scalar.activation(
                        e16[:],
                        y16[:],
                        mybir.ActivationFunctionType.Exp,
                        bias=nv1[:],
                        scale=1.0,
                    )
                    m16 = sm.tile([P, NC], FP, tag="m16")
                    nc.vector.tensor_scalar(
                        m16[:],
                        y16[:],
                        v8x[:, K - 1 : K],
                        None,
                        op0=mybir.AluOpType.is_ge,
                    )
                    ew = sm.tile([P, NC], FP, tag="ew")
                    nc.vector.tensor_mul(ew[:], e16[:], m16[:])
                    ssum = sm.tile([P, 1], FP, tag="ssum")
                    nc.vector.reduce_sum(
                        out=ssum[:], in_=ew[:], axis=mybir.AxisListType.X
                    )
                    rs = sm.tile([P, 1], FP, tag="rs")
                    nc.vector.reciprocal(rs[:], ssum[:])

                    # --- weighted sum of candidate features ---
                    nc.vector.tensor_mul(
                        g32, g32, ew[:].to_broadcast([P, NC, F])
                    )
                    half = NC
                    while half > 1:
                        half //= 2
                        nc.vector.tensor_add(
                            cg[:, 0:half, 0:F],
                            cg[:, 0:half, 0:F],
                            cg[:, half : 2 * half, 0:F],
                        )
                    ot = gpool.tile([P, F], FP, tag="ot")
                    nc.vector.tensor_scalar_mul(ot[:], cg[:, 0, 0:F], rs[:])

                    nc.sync.dma_start(out[ts(t, P), :], ot[:])

    nc.compile()
    return nc


_NC = None
LAST_RESULT = None


def _host_consts():
    ident = np.eye(P, dtype=np.float32)
    perm = np.zeros((P, 8, P), dtype=np.float32)
    for u in range(8):
        for p16 in range(16):
            perm[16 * u + p16, u, p16::16] = 1.0
    pkinit = np.tile(np.arange(NH, dtype=np.int32), (P, 1))
    return ident, perm, pkinit


def kernel(coords, positions, weights, features):
    global _NC, LAST_RESULT
    import os

    if _NC is None:
        _NC = _build_nc()

    coords = np.ascontiguousarray(coords, dtype=np.float32)
    positions = np.ascontiguousarray(positions, dtype=np.float32)
    weights = np.ascontiguousarray(weights, dtype=np.float32)
    features = np.ascontiguousarray(features, dtype=np.float32)
    ident, perm, pkinit = _host_consts()

    in_maps = [
        {
            "coords": coords[i * Q : (i + 1) * Q],
            "positions": positions,
            "weights": weights,
            "features": features,
            "ident": ident,
            "perm": perm,
            "pkinit": pkinit,
        }
        for i in range(NCORES)
    ]
    trace = bool(int(os.environ.get("KNN_TRACE", "0")))
    res = run_bass_kernel_spmd(_NC, in_maps, core_ids=list(range(NCORES)), trace=trace)
    LAST_RESULT = res
    return np.concatenate([res.results[i]["out"] for i in range(NCORES)], axis=0)



# revision 7
# speedup vs baseline: 2.7322x; 2.7322x over previous
"""Weighted-KNN (retrieval_knn) Trainium2 kernel — candidate-pruned, gather-free.

Math (per query c, over N anchors):
    sq[n]   = ||c - p_n||^2 / (w_n^2 + eps)
    top-8 smallest sq -> softmax(-sq_k / TEMP) -> weighted sum of features.

Strategy:
  HOST (numpy, in kernel()):
    * kd-split the 65536 queries into 512 spatial tiles of 128.
    * Per tile, an exact interval bound (f64) selects the candidate anchors
      that can possibly be in ANY tile query's top-8:
          T8 = 8th-smallest over anchors of max_{x in bbox} eff(x,n),
          keep n with min_{x in bbox} eff(x,n) <= T8.
      Mean ~320 candidates instead of 16384 (~39x less score work).
    * Tiles are LPT-balanced across the 8 cores (64 slots each), sorted by
      candidate count, and padded to one shared static schedule so a single
      NEFF serves all cores. Host ships per-core tables:
          qrow  [64, 384]   tile query coords (centered), rows c0|c1|c2
          gtabT [SUML, 4]   per-candidate [g0, -p'0, -p'1, -p'2],
                            g0 = -INV_TEMP/(w^2+eps)
          feat  [SUML, 72]  [features(64), 1.0, pad(7)]
  DEVICE (per tile, all engines pipelined, no dma_gather anywhere):
    * crep = partition_broadcast of the tile's query rows (Pool).
    * Scores via EXACT direct differences (same precision class as the
      reference): sq_d = Square(crep_d + (-p'_d)) on ScalarE with
      per-partition bias; y^T[j,q] = (sq0+sq1+sq2) * g0_j  (DVE).
    * PE-transpose y^T -> y[q,j] in PSUM; top-8 per query via DVE max8
      (per-512 pre-max8 + combine; exact fp32, pigeonhole-safe).
    * Mask  = (y^T >= s_bcast) (Pool), W = exp(y^T + C) * Mask
      (ScalarE exp + Pool mult; C is a global shift, cancels in softmax).
    * Feature blend as accumulating matmuls: out[q,:65] = sum_j W^T f_j,
      column 64 of feat is 1.0 so out[:,64] = Z (self-consistent softmax).
    * out = psB[:, :64] * (1/Z); DMA to DRAM; host un-permutes rows.
"""

import sys

if "/opt/trn_rl_repo" not in sys.path:
    sys.path.insert(0, "/opt/trn_rl_repo")

import numpy as np

import concourse.bacc as bacc
import concourse.bass as bass
import concourse.mybir as mybir
from concourse.bass import ts
from concourse.bass_utils import run_bass_kernel_spmd
from concourse.tile import TileContext

B, N, D, F = 65536, 16384, 3, 64
K = 8
BANDWIDTH = 0.05
TEMP = 2.0 * BANDWIDTH * BANDWIDTH  # 0.005
INV_TEMP = 1.0 / TEMP  # 200.0
EPS = 1e-8
NCORES = 8
QPC = B // NCORES  # 8192 queries per core
P = 128
NSLOT = QPC // P  # 64 tiles per core
FE = F + 8  # feat row: 64 features, ones col, 7 pad
LOOP = 1  # in-NEFF repetitions of the whole tile loop (benchmarking)

FP = mybir.dt.float32
AF = mybir.ActivationFunctionType


# ---------------------------------------------------------------- host prep
def _kd_leaves(coords):
    def split(idx, depth):
        if len(idx) == P:
            return [idx]
        ax = depth % 3
        k = len(idx) // 2
        part = np.argpartition(coords[idx, ax], k)
        return split(idx[part[:k]], depth + 1) + split(idx[part[k:]], depth + 1)

    return split(np.arange(coords.shape[0]), 0)


def prep(coords, positions, weights, features):
    """Host-side index construction. Returns (in_maps_arrays, meta)."""
    coords = np.ascontiguousarray(coords, dtype=np.float32)
    p64 = np.ascontiguousarray(positions, dtype=np.float64)
    w64 = np.ascontiguousarray(weights, dtype=np.float64)
    features = np.ascontiguousarray(features, dtype=np.float32)
    inv64 = 1.0 / (w64 * w64 + EPS)

    leaves = _kd_leaves(coords)
    ntiles = len(leaves)
    cands, centers, counts, t8s = [], [], [], []
    for lf in leaves:
        c = coords[lf].astype(np.float64)
        lo, hi = c.min(0), c.max(0)
        dmin2 = (np.clip(np.maximum(lo - p64, p64 - hi), 0, None) ** 2).sum(1)
        dmax2 = (np.maximum((p64 - lo) ** 2, (p64 - hi) ** 2)).sum(1)
        emin, emax = dmin2 * inv64, dmax2 * inv64
        t8 = np.partition(emax, K - 1)[K - 1] * (1 + 1e-4) + 1e-9
        cl = np.where(emin <= t8)[0]
        assert len(cl) >= K
        cands.append(cl)
        centers.append((lo + hi) / 2)
        counts.append(len(cl))
        t8s.append(t8)
    counts = np.array(counts)
    t8s = np.array(t8s)
    assert INV_TEMP * t8s.max() <= 160.0, t8s.max()
    cshift = float(np.clip(INV_TEMP * t8s.max() - 40.0, 0.0, 80.0))

    # LPT-balance tiles onto cores (64 slots each), sort desc by count
    order = np.argsort(-counts, kind="stable")
    loads = np.zeros(NCORES)
    slots = [[] for _ in range(NCORES)]
    for t in order:
        free = [c for c in range(NCORES) if len(slots[c]) < NSLOT]
        c = min(free, key=lambda c: loads[c])
        slots[c].append(t)
        loads[c] += counts[t]
    for c in range(NCORES):
        slots[c].sort(key=lambda t: -counts[t])
    ls = np.array(
        [[counts[slots[c][j]] for j in range(NSLOT)] for c in range(NCORES)]
    )
    sched = np.maximum(P, ((ls.max(0) + P - 1) // P) * P).astype(np.int64)
    offs = np.concatenate([[0], np.cumsum(sched)])
    suml = int(offs[-1])

    per_core = []
    outperm = []
    for c in range(NCORES):
        qrow = np.zeros((NSLOT, 3 * P), np.float32)
        gtabT = np.zeros((suml, 4), np.float32)
        gtabT[:, 0] = -1.0
        gtabT[:, 1:4] = -100.0  # pad: far away, y ~ -3e4
        feat = np.zeros((suml, FE), np.float32)
        for j in range(NSLOT):
            t = slots[c][j]
            lf, cl, ctr = leaves[t], cands[t], centers[t]
            qc = (coords[lf].astype(np.float64) - ctr).astype(np.float32)
            qrow[j, :] = qc.T.reshape(-1)
            o, n = offs[j], len(cl)
            gtabT[o : o + n, 0] = (-INV_TEMP * inv64[cl]).astype(np.float32)
            gtabT[o : o + n, 1:4] = -(p64[cl] - ctr).astype(np.float32)
            feat[o : o + n, 0:F] = features[cl]
            feat[o : o + n, F] = 1.0
            outperm.append(lf)
        per_core.append({"qrow": qrow, "gtabT": gtabT, "feat": feat})
    outperm = np.concatenate(outperm)
    inv_perm = np.empty(B, np.int64)
    inv_perm[outperm] = np.arange(B)
    meta = {
        "sched": tuple(int(x) for x in sched),
        "offs": offs,
        "suml": suml,
        "cshift": cshift,
        "inv_perm": inv_perm,
    }
    return per_core, meta


# ------------------------------------------------------------- device build
def _build_nc(sched, suml, cshift, loop=1):
    nc = bacc.Bacc("TRN2")
    qrow_in = nc.declare_dram_parameter("qrow", [NSLOT, 3 * P], FP, isOutput=False)
    gtabT_in = nc.declare_dram_parameter("gtabT", [suml, 4], FP, isOutput=False)
    feat_in = nc.declare_dram_parameter("feat", [suml, FE], FP, isOutput=False)
    ident_in = nc.declare_dram_parameter("ident", [P, P], FP, isOutput=False)
    out = nc.declare_dram_parameter("out", [QPC, F], FP, isOutput=True)

    offs = np.concatenate([[0], np.cumsum(sched)]).astype(np.int64)

    with TileContext(nc) as tc:
        with tc.tile_pool(name="const", bufs=1) as cpool:
            ident = cpool.tile([P, P], FP)
            nc.sync.dma_start(ident[:], ident_in[:])
            cbias = cpool.tile([P, 1], FP)
            nc.vector.memset(cbias[:], cshift)

            with (
                tc.tile_pool(name="io", bufs=3) as io,
                tc.tile_pool(name="work", bufs=2) as wk,
                tc.tile_pool(name="chk", bufs=4) as ck,
                tc.tile_pool(name="ps_y", bufs=3, space="PSUM") as psy,
                tc.tile_pool(name="ps_b", bufs=2, space="PSUM") as psb,
                tc.tile_pool(name="ps_s", bufs=2, space="PSUM") as pss,
            ):
                for it in range(NSLOT * loop):
                    t = it % NSLOT
                    L = int(sched[t])
                    o = int(offs[t])
                    nch = L // P  # 128-wide sub-chunks
                    ng = (L + 511) // 512  # 512-wide groups

                    # ---- loads ----
                    csrc = io.tile([1, 3 * P], FP, tag="csrc", name=f"cs{it}")
                    nc.sync.dma_start(csrc[:], qrow_in[t : t + 1, :])
                    crep = wk.tile([P, 3 * P], FP, tag="crep", name=f"cr{it}")
                    nc.gpsimd.partition_broadcast(crep[:], csrc[:])
                    pcol = io.tile([P, nch, 4], FP, tag="pcol", name=f"pc{it}")
                    nc.sync.dma_start(
                        pcol[:],
                        gtabT_in[o : o + L, :].rearrange("(c p) f -> p c f", p=P),
                    )
                    ft = io.tile([P, nch, F + 1], FP, tag="ft", name=f"ft{it}")
                    nc.sync.dma_start(
                        ft[:],
                        feat_in[o : o + L, 0 : F + 1].rearrange(
                            "(c p) f -> p c f", p=P
                        ),
                    )

                    yT = wk.tile([P, nch, P], FP, tag="yT", name=f"yT{it}")
                    e8all = wk.tile([P, 8 * ng], FP, tag="e8all", name=f"e8a{it}")

                    # ---- phase A: scores (exact direct differences) ----
                    for g in range(ng):
                        gw = min(4, nch - 4 * g)
                        psY = psy.tile([P, 512], FP, tag="psY", name=f"psY{it}_{g}")
                        for kk in range(gw):
                            ci = 4 * g + kk
                            sqa = ck.tile([P, P], FP, tag="sqa", name=f"sqa{it}_{ci}")
                            sqb = ck.tile([P, P], FP, tag="sqb", name=f"sqb{it}_{ci}")
                            nc.scalar.activation(
                                sqa[:], crep[:, 0:P], AF.Square,
                                bias=pcol[:, ci, 1:2],
                            )
                            nc.scalar.activation(
                                sqb[:], crep[:, P : 2 * P], AF.Square,
                                bias=pcol[:, ci, 2:3],
                            )
                            nc.vector.tensor_add(sqa[:], sqa[:], sqb[:])
                            nc.scalar.activation(
                                sqb[:], crep[:, 2 * P : 3 * P], AF.Square,
                                bias=pcol[:, ci, 3:4],
                            )
                            nc.vector.tensor_add(sqa[:], sqa[:], sqb[:])
                            nc.vector.tensor_scalar_mul(
                                yT[:, ci, :], sqa[:], pcol[:, ci, 0:1]
                            )
                            nc.tensor.transpose(
                                psY[:, kk * P : (kk + 1) * P], yT[:, ci, :], ident[:]
                            )
                        nc.vector.max(e8all[:, 8 * g : 8 * g + 8], psY[:, 0 : gw * P])

                    # ---- combine top-8; broadcast threshold ----
                    e8 = ck.tile([P, 8], FP, tag="e8", name=f"e8{it}")
                    nc.vector.max(e8[:], e8all[:])
                    psS = pss.tile([1, P], FP, tag="psS", name=f"psS{it}")
                    nc.tensor.transpose(psS[:], e8[:, 7:8], ident[:])
                    srow = ck.tile([1, P], FP, tag="srow", name=f"sr{it}")
                    nc.scalar.copy(srow[:], psS[:])
                    srep = wk.tile([P, P], FP, tag="srep", name=f"srp{it}")
                    nc.gpsimd.partition_broadcast(srep[:], srow[:])

                    # ---- phase B: masked exp weights + feature blend ----
                    psB = psb.tile([P, F + 1], FP, tag="psB", name=f"psB{it}")
                    for ci in range(nch):
                        msk = ck.tile([P, P], FP, tag="msk", name=f"m{it}_{ci}")
                        nc.vector.tensor_tensor(
                            out=msk[:], in0=yT[:, ci, :], in1=srep[:],
                            op=mybir.AluOpType.is_ge,
                        )
                        et = ck.tile([P, P], FP, tag="et", name=f"e{it}_{ci}")
                        nc.scalar.activation(
                            et[:], yT[:, ci, :], AF.Exp, bias=cbias[:]
                        )
                        nc.gpsimd.tensor_mul(et[:], et[:], msk[:])
                        nc.tensor.matmul(
                            psB[:],
                            et[:],
                            ft[:, ci, :],
                            start=(ci == 0),
                            stop=(ci == nch - 1),
                        )

                    # ---- normalize + store ----
                    ob = ck.tile([P, F + 1], FP, tag="ob", name=f"ob{it}")
                    nc.scalar.copy(ob[:], psB[:])
                    rs = ck.tile([P, 1], FP, tag="rs", name=f"rs{it}")
                    nc.vector.reciprocal(rs[:], ob[:, F : F + 1])
                    ot = ck.tile([P, F], FP, tag="ot", name=f"ot{it}")
                    nc.vector.tensor_scalar_mul(ot[:], ob[:, 0:F], rs[:])
                    nc.sync.dma_start(out[ts(t, P), :], ot[:])

    nc.compile()
    return nc


# ------------------------------------------------------------------ runtime
_CACHE = {}


def _get_nc(sched, suml, cshift, loop):
    key = (sched, suml, round(cshift, 6), loop)
    if key not in _CACHE:
        _CACHE[key] = _build_nc(sched, suml, cshift, loop=loop)
    return _CACHE[key]


def make_in_maps(per_core):
    ident = np.eye(P, dtype=np.float32)
    return [
        {
            "qrow": pc["qrow"],
            "gtabT": pc["gtabT"],
            "feat": pc["feat"],
            "ident": ident,
        }
        for pc in per_core
    ]


LAST_RESULT = None


def kernel(coords, positions, weights, features):
    global LAST_RESULT
    import os

    per_core, meta = prep(coords, positions, weights, features)
    nc = _get_nc(meta["sched"], meta["suml"], meta["cshift"], LOOP)
    in_maps = make_in_maps(per_core)
    trace = bool(int(os.environ.get("KNN_TRACE", "0")))
    res = run_bass_kernel_spmd(nc, in_maps, core_ids=list(range(NCORES)), trace=trace)
    LAST_RESULT = res
    full = np.concatenate([res.results[i]["out"] for i in range(NCORES)], axis=0)
    return full[meta["inv_perm"]]
